# revision 1
# baseline (speedup 1.0000x reference)
"""Trainium2 Bass kernel for ModelBLA linear state-space simulation.

Math: X_{k+1} = A X_k + B u_k ; Y_k = C X_k + D u_k ; X_0 = 0.
Shapes: u (8192, 4, 128) -> Y (8192, 4, 128), X (8192, 64, 128).

Strategy: shard TIME across the 8 cores (1024 steps each + 512-step halo of
u; A has spectral radius 0.95 so A^512 ~ 4e-12 makes the halo exact to fp32).
Per core, a 3-level blocked scan turns the sequential recurrence into dense
matmuls with R=128 realizations (x4 chunks = 512) in the PE free dimension:
  1. carry16: per 16-step chunk c, carry_c = sum_i A^(15-i) B u[16c+i]
  2. carry256 per 256-step superchunk; anchors S_super via A^256
  3. level-1 triangular scan: S_c (state at chunk starts) from carries
  4. X pass: X[16c+j] = sum_{i<j} A^(j-1-i) B u_i + A^j S_c  (single K=128)
  5. Y pass: same with C-projected impulse response.
"""
import sys
import os

for _p in ("/opt/trn_rl_repo",):
    if _p not in sys.path:
        sys.path.insert(0, _p)

import numpy as np

N, NU, R, NX, NY = 8192, 4, 128, 64, 4
NCORE = 8
T = 16                       # chunk length (steps)
NLOC = N // NCORE            # 1024 steps per core
HALO = 512                   # halo steps (pure-u history)
CH = NLOC // T               # 64 output chunks / core
CHT = (NLOC + HALO) // T     # 96 total chunks / core
NPAIR = CHT // 2             # 48 chunk pairs
SCH = 16                     # chunks per superchunk
NSUP = CH // SCH             # 4 output superchunks
HSUP = HALO // (T * SCH)     # 2 halo superchunks

F32 = None                   # set lazily (mybir.dt.float32)

_BUILD_CACHE = {}


def _build_mats(A, B_u, C_y, D_yu):
    """All packed lhsT constant matrices, fp64 -> fp32, kernel layouts."""
    A = A.astype(np.float64)
    B = B_u.astype(np.float64)
    C = C_y.astype(np.float64)
    D = D_yu.astype(np.float64)
    Ap = [np.eye(NX)]
    for _ in range(256):
        Ap.append(A @ Ap[-1])
    A16p = [Ap[16 * m] for m in range(17)]

    W16 = np.zeros((64, 64))
    for i in range(T):
        W16[i * 4:(i + 1) * 4, :] = (Ap[T - 1 - i] @ B).T
    W16pair = np.zeros((128, 128))
    W16pair[:64, :64] = W16
    W16pair[64:, 64:] = W16

    # XLh[k, r, m]: X-pass lhsT per row-tile r. K: 0-63 = U_c flat, 64-127 = S_c
    XLh = np.zeros((128, 8, 128))
    for r in range(8):
        for mm in range(128):
            j = (128 * r + mm) // 64
            st = mm % 64
            for i in range(j):
                XLh[i * 4:(i + 1) * 4, r, mm] = (Ap[j - 1 - i] @ B)[st, :]
            XLh[64:, r, mm] = Ap[j][st, :]

    # L1h[k, idx, m]: level-1 triangular tiles, idx enumerates (rt, kt<=rt)
    l1_idx = {}
    n_l1 = 0
    for rt in range(8):
        for kt in range(rt + 1):
            l1_idx[(rt, kt)] = n_l1
            n_l1 += 1
    L1h = np.zeros((128, n_l1, 128))
    for (rt, kt), idx in l1_idx.items():
        for g in range(2):
            c_out = 2 * rt + g
            for kk in range(2):
                c_in = 2 * kt + kk
                if c_in < c_out:
                    L1h[kk * 64:(kk + 1) * 64, idx, g * 64:(g + 1) * 64] = \
                        A16p[c_out - 1 - c_in].T

    C256h = np.zeros((128, 8, 64))
    for kt in range(8):
        for kk in range(2):
            C256h[kk * 64:(kk + 1) * 64, kt, :] = A16p[15 - (2 * kt + kk)].T

    P16h = np.zeros((64, 8, 128))
    for rt in range(8):
        for g in range(2):
            P16h[:, rt, g * 64:(g + 1) * 64] = A16p[2 * rt + g].T

    A256h = A16p[16].T.copy()

    YLh = np.zeros((128, 64))
    for j in range(T):
        for i in range(j):
            YLh[i * 4:(i + 1) * 4, j * 4:(j + 1) * 4] = (C @ Ap[j - 1 - i] @ B).T
        YLh[j * 4:(j + 1) * 4, j * 4:(j + 1) * 4] = D.T
        YLh[64:, j * 4:(j + 1) * 4] = (C @ Ap[j]).T

    f = np.float32
    return dict(W16pair=W16pair.astype(f), XLh=XLh.astype(f),
                L1h=L1h.astype(f), C256h=C256h.astype(f),
                P16h=P16h.astype(f), A256h=A256h.astype(f),
                YLh=YLh.astype(f)), l1_idx


def _build_bass(l1_idx):
    import concourse.bacc as bacc
    import concourse.mybir as mybir
    import concourse.tile as tile

    f32 = mybir.dt.float32
    nc = bacc.Bacc("TRN2", target_bir_lowering=False)

    u_halo = nc.declare_dram_parameter("u_halo", [NPAIR, 128, 128], f32, isOutput=False)
    W16d = nc.declare_dram_parameter("W16pair", [128, 128], f32, isOutput=False)
    XLd = nc.declare_dram_parameter("XLh", [128, 8, 128], f32, isOutput=False)
    n_l1 = len(l1_idx)
    L1d = nc.declare_dram_parameter("L1h", [128, n_l1, 128], f32, isOutput=False)
    C256d = nc.declare_dram_parameter("C256h", [128, 8, 64], f32, isOutput=False)
    P16d = nc.declare_dram_parameter("P16h", [64, 8, 128], f32, isOutput=False)
    A256d = nc.declare_dram_parameter("A256h", [64, 64], f32, isOutput=False)
    YLd = nc.declare_dram_parameter("YLh", [128, 64], f32, isOutput=False)
    Xo = nc.declare_dram_parameter("Xo", [NLOC, NX, R], f32, isOutput=True)
    Yo = nc.declare_dram_parameter("Yo", [NLOC, NY, R], f32, isOutput=True)

    with tile.TileContext(nc) as tc:
        with tc.tile_pool(name="res", bufs=1) as res, \
             tc.tile_pool(name="stage", bufs=8) as stage, \
             tc.tile_pool(name="ps", bufs=6, space="PSUM") as ps:

            # ---- resident buffers & constants ----
            u_pair = res.tile([128, NPAIR, 128], f32)
            CB = res.tile([128, CH, 128], f32)       # [0:64]=U_c, [64:128]=S_c
            carry = res.tile([128, NPAIR, 128], f32)
            c256 = res.tile([64, 6, 128], f32)
            S_sup = res.tile([64, 4, 128], f32)

            W16s = res.tile([128, 128], f32)
            XLs = res.tile([128, 8, 128], f32)
            L1s = res.tile([128, n_l1, 128], f32)
            C256s = res.tile([128, 8, 64], f32)
            P16s = res.tile([64, 8, 128], f32)
            A256s = res.tile([64, 64], f32)
            YLs = res.tile([128, 64], f32)

            nc.sync.dma_start(out=W16s[:], in_=W16d[:])
            nc.sync.dma_start(out=XLs[:], in_=XLd[:])
            nc.sync.dma_start(out=L1s[:], in_=L1d[:])
            nc.sync.dma_start(out=C256s[:], in_=C256d[:])
            nc.sync.dma_start(out=P16s[:], in_=P16d[:])
            nc.sync.dma_start(out=A256s[:], in_=A256d[:])
            nc.sync.dma_start(out=YLs[:], in_=YLd[:])
            for pi in range(NPAIR):
                nc.sync.dma_start(out=u_pair[:, pi, :], in_=u_halo[pi])

            # copy U halves into CB (chunk layout) for the X/Y-pass rhs
            nc.vector.tensor_copy(CB[0:64, 0::2, :], u_pair[0:64, 16:48, :])
            nc.vector.tensor_copy(CB[0:64, 1::2, :], u_pair[64:128, 16:48, :])

            _eng = [nc.vector, nc.scalar]

            def cp(i, out, in_):
                e = _eng[i % 2]
                if e is nc.scalar:
                    e.copy(out=out, in_=in_)
                else:
                    e.tensor_copy(out=out, in_=in_)

            # ---- stage 1: carry16 (12 matmuls over 48 pairs) ----
            for g in range(12):
                pc = ps.tile([128, 4, 128], f32, tag="ps")
                nc.tensor.matmul(pc[:], W16s[:], u_pair[:, 4 * g:4 * g + 4, :],
                                 start=True, stop=True)
                cp(g, carry[:, 4 * g:4 * g + 4, :], pc[:])

            # ---- stage 2: carry256 per superchunk (6 supers in 2 batches) ----
            for s0, ns in ((0, 4), (4, 2)):
                pcc = ps.tile([64, 4, 128], f32, tag="ps")
                for kt in range(8):
                    rhs = carry[:, 8 * s0 + kt: 8 * (s0 + ns): 8, :]
                    nc.tensor.matmul(pcc[0:64, 0:ns, :], C256s[:, kt, :], rhs,
                                     start=(kt == 0), stop=(kt == 7))
                cp(s0, c256[:, s0:s0 + ns, :], pcc[0:64, 0:ns, :])

            # ---- stage 3: anchors S_super_b = c256[b+1] + A256 @ c256[b] ----
            for b in range(4):
                pa = ps.tile([64, 128], f32, tag="ps")
                nc.tensor.matmul(pa[:], A256s[:], c256[:, b, :],
                                 start=True, stop=True)
                nc.vector.tensor_add(S_sup[:, b, :], pa[:], c256[:, b + 1, :])

            # ---- stage 4: level-1 triangular scan -> S_c into CB[64:] ----
            for rt in range(8):
                pl = ps.tile([128, 4, 128], f32, tag="ps")
                nc.tensor.matmul(pl[:], P16s[:, rt, :], S_sup[:, 0:4, :],
                                 start=True, stop=False)
                for kt in range(rt + 1):
                    rhs = carry[:, 16 + kt: 48: 8, :]
                    nc.tensor.matmul(pl[:], L1s[:, l1_idx[(rt, kt)], :], rhs,
                                     start=False, stop=(kt == rt))
                cp(0, CB[64:128, 2 * rt::16, :], pl[0:64, :, :])
                cp(1, CB[64:128, 2 * rt + 1::16, :], pl[64:128, :, :])

            # ---- stage 5: X pass ----
            nmm = 0
            for r in range(8):
                for g in range(16):
                    px = ps.tile([128, 4, 128], f32, tag="ps")
                    nc.tensor.matmul(px[:], XLs[:, r, :], CB[:, 4 * g:4 * g + 4, :],
                                     start=True, stop=True)
                    sx = stage.tile([128, 4, 128], f32, tag="st")
                    cp(nmm, sx[:], px[:])
                    nmm += 1
                    for q in range(4):
                        k0 = T * (4 * g + q) + 2 * r
                        nc.sync.dma_start(out=Xo[k0:k0 + 2, :, :], in_=sx[:, q, :])

            # ---- stage 6: Y pass ----
            for g in range(16):
                pyt = ps.tile([64, 4, 128], f32, tag="ps")
                nc.tensor.matmul(pyt[:], YLs[:], CB[:, 4 * g:4 * g + 4, :],
                                 start=True, stop=True)
                sy = stage.tile([64, 4, 128], f32, tag="st")
                cp(g, sy[:], pyt[:])
                for q in range(4):
                    k0 = T * (4 * g + q)
                    nc.sync.dma_start(out=Yo[k0:k0 + 16, :, :], in_=sy[:, q, :])

    nc.compile()
    return nc


def _get_built(A, B_u, C_y, D_yu):
    key = "nc"
    if key not in _BUILD_CACHE:
        mats, l1_idx = _build_mats(A, B_u, C_y, D_yu)
        nc = _build_bass(l1_idx)
        _BUILD_CACHE[key] = (nc, mats)
    return _BUILD_CACHE[key]


def kernel(u, A, B_u, C_y, D_yu):
    from concourse.bass_utils import run_bass_kernel_spmd

    u = np.ascontiguousarray(np.asarray(u, dtype=np.float32))
    nc, mats = _get_built(np.asarray(A), np.asarray(B_u),
                          np.asarray(C_y), np.asarray(D_yu))

    upad = np.concatenate(
        [np.zeros((HALO, NU, R), np.float32), u], axis=0)
    in_maps = []
    for core in range(NCORE):
        k0 = core * NLOC
        uh = np.ascontiguousarray(
            upad[k0:k0 + NLOC + HALO].reshape(NPAIR, 128, 128))
        m = dict(mats)
        m["u_halo"] = uh
        in_maps.append(m)

    res = run_bass_kernel_spmd(nc, in_maps, list(range(NCORE))).results
    Y = np.concatenate([res[c]["Yo"] for c in range(NCORE)], axis=0)
    X = np.concatenate([res[c]["Xo"] for c in range(NCORE)], axis=0)
    return Y, X


# revision 2
# speedup vs baseline: 1.0396x; 1.0396x over previous
"""Trainium2 Bass kernel for ModelBLA linear state-space simulation.

Math: X_{k+1} = A X_k + B u_k ; Y_k = C X_k + D u_k ; X_0 = 0.
Shapes: u (8192, 4, 128) -> Y (8192, 4, 128), X (8192, 64, 128).

Strategy: shard TIME across the 8 cores (1024 steps each + 512-step halo of
u; A has spectral radius 0.95 so A^512 ~ 4e-12 makes the halo exact to fp32).
Per core, a 3-level blocked scan turns the sequential recurrence into dense
matmuls with R=128 realizations (x4 chunks = 512) in the PE free dimension:
  1. carry16: per 16-step chunk c, carry_c = sum_i A^(15-i) B u[16c+i]
  2. carry256 per 256-step superchunk; anchors S_super via A^256
  3. level-1 triangular scan: S_c (state at chunk starts) from carries
  4. X pass: X[16c+j] = sum_{i<j} A^(j-1-i) B u_i + A^j S_c  (single K=128)
  5. Y pass: same with C-projected impulse response.
"""
import sys
import os

for _p in ("/opt/trn_rl_repo",):
    if _p not in sys.path:
        sys.path.insert(0, _p)

import numpy as np

N, NU, R, NX, NY = 8192, 4, 128, 64, 4
NCORE = 8
T = 16                       # chunk length (steps)
NLOC = N // NCORE            # 1024 steps per core
HALO = 512                   # halo steps (pure-u history)
CH = NLOC // T               # 64 output chunks / core
CHT = (NLOC + HALO) // T     # 96 total chunks / core
NPAIR = CHT // 2             # 48 chunk pairs
SCH = 16                     # chunks per superchunk
NSUP = CH // SCH             # 4 output superchunks
HSUP = HALO // (T * SCH)     # 2 halo superchunks

F32 = None                   # set lazily (mybir.dt.float32)

_BUILD_CACHE = {}


def _build_mats(A, B_u, C_y, D_yu):
    """All packed lhsT constant matrices, fp64 -> fp32, kernel layouts."""
    A = A.astype(np.float64)
    B = B_u.astype(np.float64)
    C = C_y.astype(np.float64)
    D = D_yu.astype(np.float64)
    Ap = [np.eye(NX)]
    for _ in range(256):
        Ap.append(A @ Ap[-1])
    A16p = [Ap[16 * m] for m in range(17)]

    W16 = np.zeros((64, 64))
    for i in range(T):
        W16[i * 4:(i + 1) * 4, :] = (Ap[T - 1 - i] @ B).T
    W16pair = np.zeros((128, 128))
    W16pair[:64, :64] = W16
    W16pair[64:, 64:] = W16

    # XLh[k, r, m]: X-pass lhsT per row-tile r. K: 0-63 = U_c flat, 64-127 = S_c
    XLh = np.zeros((128, 8, 128))
    for r in range(8):
        for mm in range(128):
            j = (128 * r + mm) // 64
            st = mm % 64
            for i in range(j):
                XLh[i * 4:(i + 1) * 4, r, mm] = (Ap[j - 1 - i] @ B)[st, :]
            XLh[64:, r, mm] = Ap[j][st, :]

    # L1h[k, idx, m]: level-1 triangular tiles, idx enumerates (rt, kt<=rt)
    l1_idx = {}
    n_l1 = 0
    for rt in range(8):
        for kt in range(rt + 1):
            l1_idx[(rt, kt)] = n_l1
            n_l1 += 1
    L1h = np.zeros((128, n_l1, 128))
    for (rt, kt), idx in l1_idx.items():
        for g in range(2):
            c_out = 2 * rt + g
            for kk in range(2):
                c_in = 2 * kt + kk
                if c_in < c_out:
                    L1h[kk * 64:(kk + 1) * 64, idx, g * 64:(g + 1) * 64] = \
                        A16p[c_out - 1 - c_in].T

    C256h = np.zeros((128, 8, 64))
    for kt in range(8):
        for kk in range(2):
            C256h[kk * 64:(kk + 1) * 64, kt, :] = A16p[15 - (2 * kt + kk)].T

    P16h = np.zeros((64, 8, 128))
    for rt in range(8):
        for g in range(2):
            P16h[:, rt, g * 64:(g + 1) * 64] = A16p[2 * rt + g].T

    A256h = A16p[16].T.copy()

    YLh = np.zeros((128, 64))
    for j in range(T):
        for i in range(j):
            YLh[i * 4:(i + 1) * 4, j * 4:(j + 1) * 4] = (C @ Ap[j - 1 - i] @ B).T
        YLh[j * 4:(j + 1) * 4, j * 4:(j + 1) * 4] = D.T
        YLh[64:, j * 4:(j + 1) * 4] = (C @ Ap[j]).T

    f = np.float32
    return dict(W16pair=W16pair.astype(f), XLh=XLh.astype(f),
                L1h=L1h.astype(f), C256h=C256h.astype(f),
                P16h=P16h.astype(f), A256h=A256h.astype(f),
                YLh=YLh.astype(f)), l1_idx


def _build_bass(l1_idx):
    import concourse.bacc as bacc
    import concourse.mybir as mybir
    import concourse.tile as tile

    f32 = mybir.dt.float32
    nc = bacc.Bacc("TRN2", target_bir_lowering=False)

    u_halo = nc.declare_dram_parameter("u_halo", [NPAIR, 128, 128], f32, isOutput=False)
    W16d = nc.declare_dram_parameter("W16pair", [128, 128], f32, isOutput=False)
    XLd = nc.declare_dram_parameter("XLh", [128, 8, 128], f32, isOutput=False)
    n_l1 = len(l1_idx)
    L1d = nc.declare_dram_parameter("L1h", [128, n_l1, 128], f32, isOutput=False)
    C256d = nc.declare_dram_parameter("C256h", [128, 8, 64], f32, isOutput=False)
    P16d = nc.declare_dram_parameter("P16h", [64, 8, 128], f32, isOutput=False)
    A256d = nc.declare_dram_parameter("A256h", [64, 64], f32, isOutput=False)
    YLd = nc.declare_dram_parameter("YLh", [128, 64], f32, isOutput=False)
    Xo = nc.declare_dram_parameter("Xo", [NLOC, NX, R], f32, isOutput=True)
    Yo = nc.declare_dram_parameter("Yo", [NLOC, NY, R], f32, isOutput=True)

    with tile.TileContext(nc) as tc:
        with tc.tile_pool(name="res", bufs=1) as res, \
             tc.tile_pool(name="stage", bufs=8) as stage, \
             tc.tile_pool(name="ps", bufs=6, space="PSUM") as ps:

            # ---- resident buffers & constants ----
            u_pair = res.tile([128, NPAIR, 128], f32)
            CB = res.tile([128, CH, 128], f32)       # [0:64]=U_c, [64:128]=S_c
            carry = res.tile([128, NPAIR, 128], f32)
            c256 = res.tile([64, 6, 128], f32)
            S_sup = res.tile([64, 4, 128], f32)

            W16s = res.tile([128, 128], f32)
            XLs = res.tile([128, 8, 128], f32)
            L1s = res.tile([128, n_l1, 128], f32)
            C256s = res.tile([128, 8, 64], f32)
            P16s = res.tile([64, 8, 128], f32)
            A256s = res.tile([64, 64], f32)
            YLs = res.tile([128, 64], f32)

            nc.sync.dma_start(out=W16s[:], in_=W16d[:])
            nc.sync.dma_start(out=XLs[:], in_=XLd[:])
            nc.sync.dma_start(out=L1s[:], in_=L1d[:])
            nc.sync.dma_start(out=C256s[:], in_=C256d[:])
            nc.sync.dma_start(out=P16s[:], in_=P16d[:])
            nc.sync.dma_start(out=A256s[:], in_=A256d[:])
            nc.sync.dma_start(out=YLs[:], in_=YLd[:])
            for pi in range(NPAIR):
                nc.sync.dma_start(out=u_pair[:, pi, :], in_=u_halo[pi])

            # copy U halves into CB (chunk layout) for the X/Y-pass rhs
            nc.vector.tensor_copy(CB[0:64, 0::2, :], u_pair[0:64, 16:48, :])
            nc.vector.tensor_copy(CB[0:64, 1::2, :], u_pair[64:128, 16:48, :])

            _eng = [nc.vector, nc.scalar]

            def cp(i, out, in_):
                e = _eng[i % 2]
                if e is nc.scalar:
                    e.copy(out=out, in_=in_)
                else:
                    e.tensor_copy(out=out, in_=in_)

            # ---- stage 1: carry16 (12 matmuls over 48 pairs) ----
            for g in range(12):
                pc = ps.tile([128, 4, 128], f32, tag="ps")
                nc.tensor.matmul(pc[:], W16s[:], u_pair[:, 4 * g:4 * g + 4, :],
                                 start=True, stop=True)
                cp(g, carry[:, 4 * g:4 * g + 4, :], pc[:])

            # ---- stage 2: carry256 per superchunk (6 supers in 2 batches) ----
            for s0, ns in ((0, 4), (4, 2)):
                pcc = ps.tile([64, 4, 128], f32, tag="ps")
                for kt in range(8):
                    rhs = carry[:, 8 * s0 + kt: 8 * (s0 + ns): 8, :]
                    nc.tensor.matmul(pcc[0:64, 0:ns, :], C256s[:, kt, :], rhs,
                                     start=(kt == 0), stop=(kt == 7))
                cp(s0, c256[:, s0:s0 + ns, :], pcc[0:64, 0:ns, :])

            # ---- stage 3: anchors S_super_b = c256[b+1] + A256 @ c256[b] ----
            for b in range(4):
                pa = ps.tile([64, 128], f32, tag="ps")
                nc.tensor.matmul(pa[:], A256s[:], c256[:, b, :],
                                 start=True, stop=True)
                nc.vector.tensor_add(S_sup[:, b, :], pa[:], c256[:, b + 1, :])

            # ---- stage 4: level-1 triangular scan -> S_c into CB[64:] ----
            for rt in range(8):
                pl = ps.tile([128, 4, 128], f32, tag="ps")
                nc.tensor.matmul(pl[:], P16s[:, rt, :], S_sup[:, 0:4, :],
                                 start=True, stop=False)
                for kt in range(rt + 1):
                    rhs = carry[:, 16 + kt: 48: 8, :]
                    nc.tensor.matmul(pl[:], L1s[:, l1_idx[(rt, kt)], :], rhs,
                                     start=False, stop=(kt == rt))
                cp(0, CB[64:128, 2 * rt::16, :], pl[0:64, :, :])
                cp(1, CB[64:128, 2 * rt + 1::16, :], pl[64:128, :, :])

            # ---- stage 5: X pass ----
            # Xo steps decompose as k = 64g + 16q + 2r + j; one merged DMA
            # per (r, g) tile with dst dims ordered (j, st, q, r) to match
            # the (partition=(j,st), free=(q,r)) source linearization.
            XoR = Xo.rearrange("(g q rr j) st r -> g rr j st q r", g=16, q=4, j=2)
            _dmae = [nc.sync, nc.scalar]
            nmm = 0
            for r in range(8):
                for g in range(16):
                    px = ps.tile([128, 4, 128], f32, tag="ps")
                    nc.tensor.matmul(px[:], XLs[:, r, :], CB[:, 4 * g:4 * g + 4, :],
                                     start=True, stop=True)
                    sx = stage.tile([128, 4, 128], f32, tag="st")
                    nc.vector.tensor_copy(sx[:], px[:])
                    _dmae[nmm % 2].dma_start(out=XoR[g, r], in_=sx[:])
                    nmm += 1

            # ---- stage 6: Y pass ----
            YoR = Yo.rearrange("(g q j) ny r -> g (j ny) q r", g=16, q=4, j=16)
            for g in range(16):
                pyt = ps.tile([64, 4, 128], f32, tag="ps")
                nc.tensor.matmul(pyt[:], YLs[:], CB[:, 4 * g:4 * g + 4, :],
                                 start=True, stop=True)
                sy = stage.tile([64, 4, 128], f32, tag="st")
                nc.vector.tensor_copy(sy[:], pyt[:])
                _dmae[g % 2].dma_start(out=YoR[g], in_=sy[:])

    nc.compile()
    return nc


def _get_built(A, B_u, C_y, D_yu):
    key = "nc"
    if key not in _BUILD_CACHE:
        mats, l1_idx = _build_mats(A, B_u, C_y, D_yu)
        nc = _build_bass(l1_idx)
        _BUILD_CACHE[key] = (nc, mats)
    return _BUILD_CACHE[key]


def kernel(u, A, B_u, C_y, D_yu):
    from concourse.bass_utils import run_bass_kernel_spmd

    u = np.ascontiguousarray(np.asarray(u, dtype=np.float32))
    nc, mats = _get_built(np.asarray(A), np.asarray(B_u),
                          np.asarray(C_y), np.asarray(D_yu))

    upad = np.concatenate(
        [np.zeros((HALO, NU, R), np.float32), u], axis=0)
    in_maps = []
    for core in range(NCORE):
        k0 = core * NLOC
        uh = np.ascontiguousarray(
            upad[k0:k0 + NLOC + HALO].reshape(NPAIR, 128, 128))
        m = dict(mats)
        m["u_halo"] = uh
        in_maps.append(m)

    res = run_bass_kernel_spmd(nc, in_maps, list(range(NCORE))).results
    Y = np.concatenate([res[c]["Yo"] for c in range(NCORE)], axis=0)
    X = np.concatenate([res[c]["Xo"] for c in range(NCORE)], axis=0)
    return Y, X
